# revision 30
# baseline (speedup 1.0000x reference)
"""Trainium2 Bass kernel for nn_Blur (upfirdn2d 4x4 blur, pad=(2,1)).

Formulation: out[i,j] = sum_{p,q} Kf[p,q] * x[i+p-2, j+q-2]   (Kf = flip(kernel2d))

The correctness gate is rel_err < 2e-2, so both input and output travel as
plain bf16 (~2^-9 rounding each) instead of the exact hi+lo / fp32 pair:
HBM traffic halves to 2 B/elem each way (16.8 MB per core), which is the
bf16 floor.  Weights are exact in bf16 ({1,3,9}/64, <=4 mantissa bits).

To keep the PE contraction at K=128 with a single bf16 input stream, the
input is packed BY COLUMN PARITY: SBUF partition = (s, h) where s = w mod 2,
free = (image g, half-col jo), w = 2*jo + s.  For output column j = 2*jo + t,
the 16 conv taps regroup into 5 matmul windows over half-width (32-col)
rhs slices:

  even (t=0):  D0  [K=128] lhsT=(B2|s0, B3|s1)  full jo        (start)
               Dm1 [K=128] lhsT=(B0|s0, B1|s1)  jo>=1          (stop)
  odd  (t=1):  D0  [K=128] lhsT=(B1|s0, B2|s1)  full jo        (start)
               Dm1 [K=128] lhsT=(0|s0, B0|s1)   jo>=1
               Dp1 [K=128] lhsT=(B3|s0, 0|s1)   jo<=30         (stop)

where B_q[h,i] = Kf[h-i+2, q] is the banded H-conv matrix for W-tap q.
(The two edge windows are zero-padded to K=128: matmul cost depends only
on the rhs free size, and row-offset tile_positions are unproven in this
stack.)  Per 32-image sub-batch and PE column half that is 2512 PE cycles
vs 4096 for the 4-tap K=64 alternative.  Two column groups (16 images
each) run concurrently on disjoint PE column halves (tile col 0 / 64).

Per core: 16 big batches of 64 images; DMA tiles [128, 2048] bf16 (4 KB
power-of-2 partition lines -- a non-power-of-2 line leaves a tail packet
that costs ~33% of DMA engine throughput).  PSUM accumulates fp32 exactly;
the psum->sbuf bf16 cast is split 3:1 between DVE and Activation so no
single engine exceeds the ~1.05 us/sub-batch PE floor.  Warmup matmuls
bracket the pipeline because the whole clock domain (PE *and* DMA
engines) gates to half speed until ~3.4 us of sustained PE activity, and
re-gates ~2-3 us after the PE idles.

Sharding: the 16*512 = 8192 independent (n,c) images split into 8
contiguous slabs of 1024 images, one per NeuronCore (data-parallel).
"""

import ml_dtypes
import numpy as np

import concourse.bacc as bacc
import concourse.bass as bass
import concourse.mybir as mybir
import concourse.tile as tile
from concourse.bass_utils import run_bass_kernel_spmd

N_CORES = 8
IMG = 64                      # H = W
HALF = 32                     # half-cols per image (W / 2)
N_IMAGES = 16 * 512           # 8192
PER_CORE = N_IMAGES // N_CORES  # 1024
SB_IMGS = 32                  # images per sub-batch
N_SB = PER_CORE // SB_IMGS    # 32
N_BB = N_SB // 2              # 16 big batches: DMA granularity = 2 sub-batches
BB_W = 2 * SB_IMGS * HALF     # 2048 cols per big-batch tile (4 KB lines)
CG_IMGS = SB_IMGS // 2        # 16 images per PE column group
CG_W = CG_IMGS * HALF         # 512 free cols per column group
IN_W = 2 * CG_W               # 1024: exact 2048 B partition lines (a non-power-
                              # of-2 line leaves an 8-16B tail packet per line
                              # that costs ~33% of DMA engine throughput)
OUT_W = 2 * CG_W              # 1024: even | odd
DT = mybir.dt.float32
IN_DT = mybir.dt.bfloat16
NP_IN = ml_dtypes.bfloat16

LAST_RESULTS = None  # BassKernelResults of the most recent run (for test.py)


def _build_weights(kernel2d: np.ndarray) -> np.ndarray:
    """[128, 320] bf16 lhsT bank (all windows K=128; zero rows are inert).

    B_q[h, i] = Kf[h-i+2, q] (banded H-conv for W-tap q).
    cols   0- 63: even D0   (B2 on s0 rows, B3 on s1 rows)
    cols  64-127: even Dm1  (B0 on s0, B1 on s1)
    cols 128-191: odd  D0   (B1 on s0, B2 on s1)
    cols 192-255: odd  Dm1  (0 on s0, B0 on s1)
    cols 256-319: odd  Dp1  (B3 on s0, 0 on s1)
    """
    kf = np.flip(np.asarray(kernel2d, dtype=np.float64), (0, 1))
    B = np.zeros((4, IMG, IMG))
    for q in range(4):
        for i in range(IMG):
            for p in range(4):
                h = i + p - 2
                if 0 <= h < IMG:
                    B[q, h, i] = kf[p, q]
    wts = np.zeros((128, 320), dtype=NP_IN)
    wts[0:64, 0:64] = B[2]
    wts[64:128, 0:64] = B[3]
    wts[0:64, 64:128] = B[0]
    wts[64:128, 64:128] = B[1]
    wts[0:64, 128:192] = B[1]
    wts[64:128, 128:192] = B[2]
    wts[64:128, 192:256] = B[0]
    wts[0:64, 256:320] = B[3]
    return wts


def _bass_module() -> bass.Bass:
    nc = bacc.Bacc(
        "TRN2",
        target_bir_lowering=False,
        debug=False,
        num_devices=N_CORES,
    )
    x_d = nc.dram_tensor("x", [N_BB, 128, BB_W], IN_DT, kind="ExternalInput")
    w_d = nc.dram_tensor("wts", [128, 320], IN_DT, kind="ExternalInput")
    o_d = nc.dram_tensor("out", [N_BB, 128, BB_W], IN_DT, kind="ExternalOutput")

    with tile.TileContext(nc) as tc:
        with (
            tc.tile_pool(name="const", bufs=1) as cpool,
            tc.tile_pool(name="inp", bufs=8) as ipool,
            tc.tile_pool(name="outp", bufs=6) as opool,
            tc.tile_pool(name="psum", bufs=8, space="PSUM") as ppool,
        ):
            w_tile = cpool.tile([128, 320], IN_DT)
            # weights ride the scalar (Activation) HWDGE queue so they do not
            # serialize ahead of the first input tile on the sync queue.
            nc.scalar.dma_start(w_tile[:], w_d[:])

            # HAM warmup: the PE exits the framework preamble at ~7.2 us
            # while the first input tile is still in flight (lands ~10.5 us
            # at the gated half clock).  Dummy matmuls fill that idle window
            # and complete the ~3.4 us sustained-activity ramp, so the real
            # matmuls start at 2.4 GHz.
            dummy = cpool.tile([128, 512], IN_DT, tag="warm_sbuf")
            nc.vector.memset(dummy[:], 0.0)
            # warmup psum shares the "ps" tag/shape so the pool fits 4 real
            # buffers in the 8 PSUM banks (4 x 2 banks, slot 0 doubles as
            # the warmup target before the rotation reaches it again).
            warm_ps = ppool.tile([128, OUT_W], DT, tag="ps", bufs=4)
            # many SMALL warmups (128 cols, ~160 ns each at the gated clock):
            # the first input tile's landing time varies run to run, and any
            # PE idle gap longer than the HAM hysteresis resets the ramp
            # accumulator.  Fine granularity keeps the eventual gap tiny and
            # an overshoot costs only ~60 ns per extra matmul.
            for _ in range(24):
                nc.tensor.matmul(
                    warm_ps[:, 0:128], dummy[:, 0:128], dummy[:, 0:128],
                    start=True, stop=True
                )

            w_e0 = w_tile[:, 0:64]
            w_em1 = w_tile[:, 64:128]
            w_o0 = w_tile[:, 128:192]
            w_om1 = w_tile[:, 192:256]
            w_op1 = w_tile[:, 256:320]

            # the out-DMA for big-batch bb is emitted one iteration LATE
            # (after bb+1's casts on the same scalar queue): an in-order
            # engine queue stalls on a waiting DMA, and a stalled head
            # blocks every ACT behind it, which delays PSUM buffer release
            # and starves the PE.  One big-batch of delay makes the wait
            # already-satisfied at queue head.
            pending_out = None
            for bb in range(N_BB):
                in_tile = ipool.tile([128, BB_W], IN_DT)
                if bb == 0:
                    # first tile split in two: the whole machine runs at the
                    # gated half clock until the PE ramp fires, so the first
                    # transfer is slow -- a half tile lets sb0's matmuls
                    # start ~2 us earlier (subtile deps cover the k=0 half).
                    nc.sync.dma_start(in_tile[:, 0:OUT_W], x_d[bb][:, 0:OUT_W])
                    nc.sync.dma_start(in_tile[:, OUT_W:BB_W], x_d[bb][:, OUT_W:BB_W])
                else:
                    nc.sync.dma_start(in_tile[:], x_d[bb])
                out_tile = opool.tile([128, BB_W], IN_DT)

                for k in range(2):
                    kb = k * OUT_W
                    # one 2-bank psum tile per sub-batch: even in cols 0-511
                    # (bank 0), odd in 512-1023 (bank 1)
                    ps = ppool.tile([128, OUT_W], DT, tag="ps", bufs=4)
                    ps_e = ps[:, 0:CG_W]
                    ps_o = ps[:, CG_W:OUT_W]

                    # window list per column group; interleave cg inner so
                    # the two PE column halves stay concurrently busy.
                    wins = []
                    for cg in range(2):
                        wins.append((ps_e, (0, HALF), w_e0, 0, HALF, True, False, cg))
                        wins.append((ps_e, (1, HALF - 1), w_em1, 0, HALF - 1, False, True, cg))
                        wins.append((ps_o, (0, HALF), w_o0, 0, HALF, True, False, cg))
                        wins.append((ps_o, (1, HALF - 1), w_om1, 0, HALF - 1, False, False, cg))
                        wins.append((ps_o, (0, HALF - 1), w_op1, 1, HALF - 1, False, True, cg))
                    order = [0, 5, 1, 6, 2, 7, 3, 8, 4, 9]  # cg0/cg1 interleaved
                    for idx in order:
                        psr, (jlo, jn), lhsT, rlo, rlen, st, sp, cg = wins[idx]
                        cgb = kb + cg * CG_W
                        rhs = in_tile[:, cgb:cgb + CG_W].rearrange(
                            "p (g w) -> p g w", w=HALF
                        )[:, :, rlo:rlo + rlen]
                        out_ap = psr[cg * 64:(cg + 1) * 64, :].rearrange(
                            "p (g w) -> p g w", w=HALF
                        )[:, :, jlo:jlo + jn]
                        nc.tensor.matmul(
                            out_ap,
                            lhsT,
                            rhs,
                            start=st,
                            stop=sp,
                            tile_position=(0, cg * 64),
                            skip_group_check=True,
                        )

                    # psum->sbuf bf16 cast, split 3:1 across DVE and
                    # Activation: one DVE cast of the full [128,1024] takes
                    # 1214 ns -- more than the 1047 ns PE floor per
                    # sub-batch, so it would pace the pipeline.  (gpsimd
                    # cannot read PSUM on hardware.)
                    nc.vector.tensor_copy(out_tile[:, kb:kb + 768], ps[:, 0:768])
                    nc.scalar.copy(out_tile[:, kb + 768:kb + OUT_W], ps[:, 768:OUT_W])
                    if bb == N_BB - 1:
                        # stream the final big-batch out per sub-batch on
                        # alternating queues: its drain overlaps the last
                        # compute instead of serializing after it.
                        eng = nc.scalar if k == 0 else nc.sync
                        eng.dma_start(o_d[bb][:, kb:kb + OUT_W],
                                      out_tile[:, kb:kb + OUT_W])

                if pending_out is not None:
                    if bb == N_BB - 1:
                        # split the two final out tiles across both HWDGE
                        # queues: a lone 524 KB transfer drains ~3 us on one
                        # queue right at the tail.
                        nc.scalar.dma_start(o_d[bb - 1][:, 0:OUT_W],
                                            pending_out[:, 0:OUT_W])
                        nc.sync.dma_start(o_d[bb - 1][:, OUT_W:BB_W],
                                          pending_out[:, OUT_W:BB_W])
                    else:
                        nc.scalar.dma_start(o_d[bb - 1], pending_out[:])
                pending_out = out_tile

            # end-warmup: the clock domain gates to half speed ~3 us after
            # the PE goes idle, which halves the final out-DMA drain rate.
            # Six dummy matmuls (~2.3 us) keep the clock up until the last
            # transfer completes.  Fresh pool tile: slot rotation makes it
            # wait only on an already-drained sub-batch, not the final one.
            warm_end = ppool.tile([128, OUT_W], DT, tag="ps", bufs=4)
            for _ in range(10):
                nc.tensor.matmul(
                    warm_end[:, 0:512], dummy[:, 0:128], dummy[:], start=True, stop=True
                )
    nc.compile()
    return nc


def _host_pack(x: np.ndarray) -> np.ndarray:
    """FULL x (8192,64,64) f32 -> [N_CORES, N_SB, 128, IN_W] bf16.

    Partition dim = (s = w&1, h); free dim = (g: 32 images, jo: 32)."""
    xb = x.astype(NP_IN)
    v = xb.reshape(N_CORES, N_SB, SB_IMGS, IMG, HALF, 2)
    v = v.transpose(0, 1, 5, 3, 2, 4)  # [core, sb, s, h, g, jo]
    flat = v.reshape(N_CORES, N_SB, 128, SB_IMGS * HALF)
    # regroup pairs of sub-batches into big-batch tiles [128, 2048]
    bb = flat.reshape(N_CORES, N_BB, 2, 128, OUT_W).transpose(0, 1, 3, 2, 4)
    return np.ascontiguousarray(bb.reshape(N_CORES, N_BB, 128, BB_W))


def _host_unpack(tiles: np.ndarray) -> np.ndarray:
    """[N_CORES, N_SB, 128, OUT_W] -> (8192, 64, 64) f32.

    Partition = (cg, i); free = (t: even/odd, g: 16, jo: 32); j = 2*jo+t;
    img = sb*32 + cg*16 + g."""
    # split big-batch tiles [N_BB, 128, 2048] back into sub-batches
    tiles = tiles.reshape(N_CORES, N_BB, 128, 2, OUT_W).transpose(0, 1, 3, 2, 4)
    v = tiles.reshape(N_CORES, N_SB, 2, IMG, 2, CG_IMGS, HALF)
    v = v.transpose(0, 1, 2, 5, 3, 6, 4)  # [core, sb, cg, g, i, jo, t]
    return np.ascontiguousarray(v).reshape(N_IMAGES, IMG, IMG)


def kernel(x: np.ndarray, kernel: np.ndarray, _trace: bool = False) -> np.ndarray:
    global LAST_RESULTS
    x = np.ascontiguousarray(np.asarray(x, dtype=np.float32))
    n, c, h, w = x.shape
    assert (n, c, h, w) == (16, 512, 64, 64), x.shape

    shards = _host_pack(x.reshape(N_IMAGES, IMG, IMG))
    wts = _build_weights(kernel)
    in_maps = [{"x": shards[i], "wts": wts} for i in range(N_CORES)]

    nc = _bass_module()
    results = run_bass_kernel_spmd(
        nc, in_maps, core_ids=list(range(N_CORES)), trace=_trace
    )
    LAST_RESULTS = results

    tiles = np.stack([r["out"] for r in results.results]).astype(np.float32)
    out = _host_unpack(tiles)
    return np.ascontiguousarray(out.reshape(n, c, h, w)).astype(np.float32)


# revision 31
# speedup vs baseline: 1.0581x; 1.0581x over previous
"""Trainium2 Bass kernel for nn_Blur (upfirdn2d 4x4 blur, pad=(2,1)).

Formulation: out[i,j] = sum_{p,q} Kf[p,q] * x[i+p-2, j+q-2]   (Kf = flip(kernel2d))

The correctness gate is rel_err < 2e-2, so both input and output travel as
plain bf16 (~2^-9 rounding each) instead of the exact hi+lo / fp32 pair:
HBM traffic halves to 2 B/elem each way (16.8 MB per core), which is the
bf16 floor.  Weights are exact in bf16 ({1,3,9}/64, <=4 mantissa bits).

To keep the PE contraction at K=128 with a single bf16 input stream, the
input is packed BY COLUMN PARITY: SBUF partition = (s, h) where s = w mod 2,
free = (image g, half-col jo), w = 2*jo + s.  For output column j = 2*jo + t,
the 16 conv taps regroup into 5 matmul windows over half-width (32-col)
rhs slices:

  even (t=0):  D0  [K=128] lhsT=(B2|s0, B3|s1)  full jo        (start)
               Dm1 [K=128] lhsT=(B0|s0, B1|s1)  jo>=1          (stop)
  odd  (t=1):  D0  [K=128] lhsT=(B1|s0, B2|s1)  full jo        (start)
               Dm1 [K=128] lhsT=(0|s0, B0|s1)   jo>=1
               Dp1 [K=128] lhsT=(B3|s0, 0|s1)   jo<=30         (stop)

where B_q[h,i] = Kf[h-i+2, q] is the banded H-conv matrix for W-tap q.
(The two edge windows are zero-padded to K=128: matmul cost depends only
on the rhs free size, and row-offset tile_positions are unproven in this
stack.)  Per 32-image sub-batch and PE column half that is 2512 PE cycles
vs 4096 for the 4-tap K=64 alternative.  Two column groups (16 images
each) run concurrently on disjoint PE column halves (tile col 0 / 64).

Per core: 16 big batches of 64 images; DMA tiles [128, 2048] bf16 (4 KB
power-of-2 partition lines -- a non-power-of-2 line leaves a tail packet
that costs ~33% of DMA engine throughput).  PSUM accumulates fp32 exactly;
the psum->sbuf bf16 cast is split 3:1 between DVE and Activation so no
single engine exceeds the ~1.05 us/sub-batch PE floor.  Warmup matmuls
bracket the pipeline because the whole clock domain (PE *and* DMA
engines) gates to half speed until ~3.4 us of sustained PE activity, and
re-gates ~2-3 us after the PE idles.

Sharding: the 16*512 = 8192 independent (n,c) images split into 8
contiguous slabs of 1024 images, one per NeuronCore (data-parallel).
"""

import ml_dtypes
import numpy as np

import concourse.bacc as bacc
import concourse.bass as bass
import concourse.mybir as mybir
import concourse.tile as tile
from concourse.bass_utils import run_bass_kernel_spmd

N_CORES = 8
IMG = 64                      # H = W
HALF = 32                     # half-cols per image (W / 2)
N_IMAGES = 16 * 512           # 8192
PER_CORE = N_IMAGES // N_CORES  # 1024
SB_IMGS = 32                  # images per sub-batch
N_SB = PER_CORE // SB_IMGS    # 32
N_BB = N_SB // 2              # 16 big batches: DMA granularity = 2 sub-batches
BB_W = 2 * SB_IMGS * HALF     # 2048 cols per big-batch tile (4 KB lines)
CG_IMGS = SB_IMGS // 2        # 16 images per PE column group
CG_W = CG_IMGS * HALF         # 512 free cols per column group
IN_W = 2 * CG_W               # 1024: exact 2048 B partition lines (a non-power-
                              # of-2 line leaves an 8-16B tail packet per line
                              # that costs ~33% of DMA engine throughput)
OUT_W = 2 * CG_W              # 1024: even | odd
DT = mybir.dt.float32
IN_DT = mybir.dt.bfloat16
NP_IN = ml_dtypes.bfloat16

LAST_RESULTS = None  # BassKernelResults of the most recent run (for test.py)


def _build_weights(kernel2d: np.ndarray) -> np.ndarray:
    """[128, 320] bf16 lhsT bank (all windows K=128; zero rows are inert).

    B_q[h, i] = Kf[h-i+2, q] (banded H-conv for W-tap q).
    cols   0- 63: even D0   (B2 on s0 rows, B3 on s1 rows)
    cols  64-127: even Dm1  (B0 on s0, B1 on s1)
    cols 128-191: odd  D0   (B1 on s0, B2 on s1)
    cols 192-255: odd  Dm1  (0 on s0, B0 on s1)
    cols 256-319: odd  Dp1  (B3 on s0, 0 on s1)
    """
    kf = np.flip(np.asarray(kernel2d, dtype=np.float64), (0, 1))
    B = np.zeros((4, IMG, IMG))
    for q in range(4):
        for i in range(IMG):
            for p in range(4):
                h = i + p - 2
                if 0 <= h < IMG:
                    B[q, h, i] = kf[p, q]
    wts = np.zeros((128, 320), dtype=NP_IN)
    wts[0:64, 0:64] = B[2]
    wts[64:128, 0:64] = B[3]
    wts[0:64, 64:128] = B[0]
    wts[64:128, 64:128] = B[1]
    wts[0:64, 128:192] = B[1]
    wts[64:128, 128:192] = B[2]
    wts[64:128, 192:256] = B[0]
    wts[0:64, 256:320] = B[3]
    return wts


def _bass_module() -> bass.Bass:
    nc = bacc.Bacc(
        "TRN2",
        target_bir_lowering=False,
        debug=False,
        num_devices=N_CORES,
    )
    x_d = nc.dram_tensor("x", [N_BB, 128, BB_W], IN_DT, kind="ExternalInput")
    w_d = nc.dram_tensor("wts", [128, 320], IN_DT, kind="ExternalInput")
    o_d = nc.dram_tensor("out", [N_BB, 128, BB_W], IN_DT, kind="ExternalOutput")

    with tile.TileContext(nc) as tc:
        with (
            tc.tile_pool(name="const", bufs=1) as cpool,
            tc.tile_pool(name="inp", bufs=8) as ipool,
            tc.tile_pool(name="outp", bufs=6) as opool,
            tc.tile_pool(name="psum", bufs=8, space="PSUM") as ppool,
        ):
            w_tile = cpool.tile([128, 320], IN_DT)
            # weights ride the scalar (Activation) HWDGE queue so they do not
            # serialize ahead of the first input tile on the sync queue.
            nc.scalar.dma_start(w_tile[:], w_d[:])

            # HAM warmup: the PE exits the framework preamble at ~7.2 us
            # while the first input tile is still in flight (lands ~10.5 us
            # at the gated half clock).  Dummy matmuls fill that idle window
            # and complete the ~3.4 us sustained-activity ramp, so the real
            # matmuls start at 2.4 GHz.
            dummy = cpool.tile([128, 512], IN_DT, tag="warm_sbuf")
            nc.vector.memset(dummy[:], 0.0)
            # warmup psum shares the "ps" tag/shape so the pool fits 4 real
            # buffers in the 8 PSUM banks (4 x 2 banks, slot 0 doubles as
            # the warmup target before the rotation reaches it again).
            warm_ps = ppool.tile([128, OUT_W], DT, tag="ps", bufs=4)
            # many SMALL warmups (128 cols, ~160 ns each at the gated clock):
            # the first input tile's landing time varies run to run, and any
            # PE idle gap longer than the HAM hysteresis resets the ramp
            # accumulator.  Fine granularity keeps the eventual gap tiny and
            # an overshoot costs only ~60 ns per extra matmul.
            for _ in range(12):
                nc.tensor.matmul(
                    warm_ps[:, 0:128], dummy[:, 0:128], dummy[:, 0:128],
                    start=True, stop=True
                )

            w_e0 = w_tile[:, 0:64]
            w_em1 = w_tile[:, 64:128]
            w_o0 = w_tile[:, 128:192]
            w_om1 = w_tile[:, 192:256]
            w_op1 = w_tile[:, 256:320]

            # the out-DMA for big-batch bb is emitted one iteration LATE
            # (after bb+1's casts on the same scalar queue): an in-order
            # engine queue stalls on a waiting DMA, and a stalled head
            # blocks every ACT behind it, which delays PSUM buffer release
            # and starves the PE.  One big-batch of delay makes the wait
            # already-satisfied at queue head.
            pending_out = None
            for bb in range(N_BB):
                in_tile = ipool.tile([128, BB_W], IN_DT)
                if bb == 0:
                    # first tile split in quarters: the whole machine runs at
                    # the gated half clock until the PE ramp fires, so the
                    # first transfer is slow -- a quarter tile (one column
                    # group) lets sb0's cg0 matmuls start ~1.3 us earlier
                    # (subtile deps; sb0 issues cg-major below).
                    for q4 in range(4):
                        nc.sync.dma_start(in_tile[:, q4 * CG_W:(q4 + 1) * CG_W],
                                          x_d[bb][:, q4 * CG_W:(q4 + 1) * CG_W])
                else:
                    nc.sync.dma_start(in_tile[:], x_d[bb])
                out_tile = opool.tile([128, BB_W], IN_DT)

                for k in range(2):
                    kb = k * OUT_W
                    # one 2-bank psum tile per sub-batch: even in cols 0-511
                    # (bank 0), odd in 512-1023 (bank 1)
                    ps = ppool.tile([128, OUT_W], DT, tag="ps", bufs=4)
                    ps_e = ps[:, 0:CG_W]
                    ps_o = ps[:, CG_W:OUT_W]

                    # window list per column group; interleave cg inner so
                    # the two PE column halves stay concurrently busy.
                    wins = []
                    for cg in range(2):
                        wins.append((ps_e, (0, HALF), w_e0, 0, HALF, True, False, cg))
                        wins.append((ps_e, (1, HALF - 1), w_em1, 0, HALF - 1, False, True, cg))
                        wins.append((ps_o, (0, HALF), w_o0, 0, HALF, True, False, cg))
                        wins.append((ps_o, (1, HALF - 1), w_om1, 0, HALF - 1, False, False, cg))
                        wins.append((ps_o, (0, HALF - 1), w_op1, 1, HALF - 1, False, True, cg))
                    if bb == 0 and k == 0:
                        # cg-major: cg0's windows depend only on the first
                        # quarter tile, so they start before cg1's data lands
                        order = [0, 1, 2, 3, 4, 5, 6, 7, 8, 9]
                    else:
                        order = [0, 5, 1, 6, 2, 7, 3, 8, 4, 9]  # cg0/cg1 interleaved
                    for idx in order:
                        psr, (jlo, jn), lhsT, rlo, rlen, st, sp, cg = wins[idx]
                        cgb = kb + cg * CG_W
                        rhs = in_tile[:, cgb:cgb + CG_W].rearrange(
                            "p (g w) -> p g w", w=HALF
                        )[:, :, rlo:rlo + rlen]
                        out_ap = psr[cg * 64:(cg + 1) * 64, :].rearrange(
                            "p (g w) -> p g w", w=HALF
                        )[:, :, jlo:jlo + jn]
                        nc.tensor.matmul(
                            out_ap,
                            lhsT,
                            rhs,
                            start=st,
                            stop=sp,
                            tile_position=(0, cg * 64),
                            skip_group_check=True,
                        )

                    # psum->sbuf bf16 cast, split 3:1 across DVE and
                    # Activation: one DVE cast of the full [128,1024] takes
                    # 1214 ns -- more than the 1047 ns PE floor per
                    # sub-batch, so it would pace the pipeline.  (gpsimd
                    # cannot read PSUM on hardware.)
                    nc.vector.tensor_copy(out_tile[:, kb:kb + 768], ps[:, 0:768])
                    nc.scalar.copy(out_tile[:, kb + 768:kb + OUT_W], ps[:, 768:OUT_W])
                    if bb >= N_BB - 2:
                        # stream the last two big-batches out per sub-batch
                        # on alternating queues: their drains overlap the
                        # last compute instead of serializing after it.
                        eng = nc.scalar if k == 0 else nc.sync
                        eng.dma_start(o_d[bb][:, kb:kb + OUT_W],
                                      out_tile[:, kb:kb + OUT_W])

                if pending_out is not None and bb < N_BB - 1:
                    nc.scalar.dma_start(o_d[bb - 1], pending_out[:])
                pending_out = out_tile

            # end-warmup: the clock domain gates to half speed ~3 us after
            # the PE goes idle, which halves the final out-DMA drain rate.
            # Six dummy matmuls (~2.3 us) keep the clock up until the last
            # transfer completes.  Fresh pool tile: slot rotation makes it
            # wait only on an already-drained sub-batch, not the final one.
            warm_end = ppool.tile([128, OUT_W], DT, tag="ps", bufs=4)
            for _ in range(10):
                nc.tensor.matmul(
                    warm_end[:, 0:512], dummy[:, 0:128], dummy[:], start=True, stop=True
                )
    nc.compile()
    return nc


def _host_pack(x: np.ndarray) -> np.ndarray:
    """FULL x (8192,64,64) f32 -> [N_CORES, N_SB, 128, IN_W] bf16.

    Partition dim = (s = w&1, h); free dim = (g: 32 images, jo: 32)."""
    xb = x.astype(NP_IN)
    v = xb.reshape(N_CORES, N_SB, SB_IMGS, IMG, HALF, 2)
    v = v.transpose(0, 1, 5, 3, 2, 4)  # [core, sb, s, h, g, jo]
    flat = v.reshape(N_CORES, N_SB, 128, SB_IMGS * HALF)
    # regroup pairs of sub-batches into big-batch tiles [128, 2048]
    bb = flat.reshape(N_CORES, N_BB, 2, 128, OUT_W).transpose(0, 1, 3, 2, 4)
    return np.ascontiguousarray(bb.reshape(N_CORES, N_BB, 128, BB_W))


def _host_unpack(tiles: np.ndarray) -> np.ndarray:
    """[N_CORES, N_SB, 128, OUT_W] -> (8192, 64, 64) f32.

    Partition = (cg, i); free = (t: even/odd, g: 16, jo: 32); j = 2*jo+t;
    img = sb*32 + cg*16 + g."""
    # split big-batch tiles [N_BB, 128, 2048] back into sub-batches
    tiles = tiles.reshape(N_CORES, N_BB, 128, 2, OUT_W).transpose(0, 1, 3, 2, 4)
    v = tiles.reshape(N_CORES, N_SB, 2, IMG, 2, CG_IMGS, HALF)
    v = v.transpose(0, 1, 2, 5, 3, 6, 4)  # [core, sb, cg, g, i, jo, t]
    return np.ascontiguousarray(v).reshape(N_IMAGES, IMG, IMG)


def kernel(x: np.ndarray, kernel: np.ndarray, _trace: bool = False) -> np.ndarray:
    global LAST_RESULTS
    x = np.ascontiguousarray(np.asarray(x, dtype=np.float32))
    n, c, h, w = x.shape
    assert (n, c, h, w) == (16, 512, 64, 64), x.shape

    shards = _host_pack(x.reshape(N_IMAGES, IMG, IMG))
    wts = _build_weights(kernel)
    in_maps = [{"x": shards[i], "wts": wts} for i in range(N_CORES)]

    nc = _bass_module()
    results = run_bass_kernel_spmd(
        nc, in_maps, core_ids=list(range(N_CORES)), trace=_trace
    )
    LAST_RESULTS = results

    tiles = np.stack([r["out"] for r in results.results]).astype(np.float32)
    out = _host_unpack(tiles)
    return np.ascontiguousarray(out.reshape(n, c, h, w)).astype(np.float32)


# revision 33
# speedup vs baseline: 1.0970x; 1.0368x over previous
"""Trainium2 Bass kernel for nn_Blur (upfirdn2d 4x4 blur, pad=(2,1)).

Formulation: out[i,j] = sum_{p,q} Kf[p,q] * x[i+p-2, j+q-2]   (Kf = flip(kernel2d))

The correctness gate is rel_err < 2e-2, so both input and output travel as
plain bf16 (~2^-9 rounding each) instead of the exact hi+lo / fp32 pair:
HBM traffic halves to 2 B/elem each way (16.8 MB per core), which is the
bf16 floor.  Weights are exact in bf16 ({1,3,9}/64, <=4 mantissa bits).

To keep the PE contraction at K=128 with a single bf16 input stream, the
input is packed BY COLUMN PARITY: SBUF partition = (s, h) where s = w mod 2,
free = (image g, half-col jo), w = 2*jo + s.  For output column j = 2*jo + t,
the 16 conv taps regroup into 5 matmul windows over half-width (32-col)
rhs slices:

  even (t=0):  D0  [K=128] lhsT=(B2|s0, B3|s1)  full jo        (start)
               Dm1 [K=128] lhsT=(B0|s0, B1|s1)  jo>=1          (stop)
  odd  (t=1):  D0  [K=128] lhsT=(B1|s0, B2|s1)  full jo        (start)
               Dm1 [K=128] lhsT=(0|s0, B0|s1)   jo>=1
               Dp1 [K=128] lhsT=(B3|s0, 0|s1)   jo<=30         (stop)

where B_q[h,i] = Kf[h-i+2, q] is the banded H-conv matrix for W-tap q.
(The two edge windows are zero-padded to K=128: matmul cost depends only
on the rhs free size, and row-offset tile_positions are unproven in this
stack.)  Per 32-image sub-batch and PE column half that is 2512 PE cycles
vs 4096 for the 4-tap K=64 alternative.  Two column groups (16 images
each) run concurrently on disjoint PE column halves (tile col 0 / 64).

Per core: 16 big batches of 64 images; DMA tiles [128, 2048] bf16 (4 KB
power-of-2 partition lines -- a non-power-of-2 line leaves a tail packet
that costs ~33% of DMA engine throughput).  PSUM accumulates fp32 exactly;
the psum->sbuf bf16 cast is split 3:1 between DVE and Activation so no
single engine exceeds the ~1.05 us/sub-batch PE floor.  Warmup matmuls
bracket the pipeline because the whole clock domain (PE *and* DMA
engines) gates to half speed until ~3.4 us of sustained PE activity, and
re-gates ~2-3 us after the PE idles.

Sharding: the 16*512 = 8192 independent (n,c) images split into 8
contiguous slabs of 1024 images, one per NeuronCore (data-parallel).
"""

import ml_dtypes
import numpy as np

import concourse.bacc as bacc
import concourse.bass as bass
import concourse.mybir as mybir
import concourse.tile as tile
from concourse.bass_utils import run_bass_kernel_spmd

N_CORES = 8
IMG = 64                      # H = W
HALF = 32                     # half-cols per image (W / 2)
N_IMAGES = 16 * 512           # 8192
PER_CORE = N_IMAGES // N_CORES  # 1024
SB_IMGS = 32                  # images per sub-batch
N_SB = PER_CORE // SB_IMGS    # 32
N_BB = N_SB // 2              # 16 big batches: DMA granularity = 2 sub-batches
BB_W = 2 * SB_IMGS * HALF     # 2048 cols per big-batch tile (4 KB lines)
CG_IMGS = SB_IMGS // 2        # 16 images per PE column group
CG_W = CG_IMGS * HALF         # 512 free cols per column group
IN_W = 2 * CG_W               # 1024: exact 2048 B partition lines (a non-power-
                              # of-2 line leaves an 8-16B tail packet per line
                              # that costs ~33% of DMA engine throughput)
OUT_W = 2 * CG_W              # 1024: even | odd
DT = mybir.dt.float32
IN_DT = mybir.dt.bfloat16
NP_IN = ml_dtypes.bfloat16

LAST_RESULTS = None  # BassKernelResults of the most recent run (for test.py)


def _build_weights(kernel2d: np.ndarray) -> np.ndarray:
    """[128, 320] bf16 lhsT bank (all windows K=128; zero rows are inert).

    B_q[h, i] = Kf[h-i+2, q] (banded H-conv for W-tap q).
    cols   0- 63: even D0   (B2 on s0 rows, B3 on s1 rows)
    cols  64-127: even Dm1  (B0 on s0, B1 on s1)
    cols 128-191: odd  D0   (B1 on s0, B2 on s1)
    cols 192-255: odd  Dm1  (0 on s0, B0 on s1)
    cols 256-319: odd  Dp1  (B3 on s0, 0 on s1)
    """
    kf = np.flip(np.asarray(kernel2d, dtype=np.float64), (0, 1))
    B = np.zeros((4, IMG, IMG))
    for q in range(4):
        for i in range(IMG):
            for p in range(4):
                h = i + p - 2
                if 0 <= h < IMG:
                    B[q, h, i] = kf[p, q]
    wts = np.zeros((128, 320), dtype=NP_IN)
    wts[0:64, 0:64] = B[2]
    wts[64:128, 0:64] = B[3]
    wts[0:64, 64:128] = B[0]
    wts[64:128, 64:128] = B[1]
    wts[0:64, 128:192] = B[1]
    wts[64:128, 128:192] = B[2]
    wts[64:128, 192:256] = B[0]
    wts[0:64, 256:320] = B[3]
    return wts


def _bass_module() -> bass.Bass:
    nc = bacc.Bacc(
        "TRN2",
        target_bir_lowering=False,
        debug=False,
        num_devices=N_CORES,
    )
    x_d = nc.dram_tensor("x", [N_BB, 128, BB_W], IN_DT, kind="ExternalInput")
    w_d = nc.dram_tensor("wts", [128, 320], IN_DT, kind="ExternalInput")
    o_d = nc.dram_tensor("out", [N_BB, 128, BB_W], IN_DT, kind="ExternalOutput")

    with tile.TileContext(nc) as tc:
        with (
            tc.tile_pool(name="const", bufs=1) as cpool,
            tc.tile_pool(name="inp", bufs=8) as ipool,
            tc.tile_pool(name="outp", bufs=6) as opool,
            tc.tile_pool(name="psum", bufs=8, space="PSUM") as ppool,
        ):
            w_tile = cpool.tile([128, 320], IN_DT)
            # weights ride the scalar (Activation) HWDGE queue so they do not
            # serialize ahead of the first input tile on the sync queue.
            nc.scalar.dma_start(w_tile[:], w_d[:])

            # HAM warmup: the PE exits the framework preamble at ~7.2 us
            # while the first input tile is still in flight (lands ~10.5 us
            # at the gated half clock).  Dummy matmuls fill that idle window
            # and complete the ~3.4 us sustained-activity ramp, so the real
            # matmuls start at 2.4 GHz.
            dummy = cpool.tile([128, 512], IN_DT, tag="warm_sbuf")
            nc.vector.memset(dummy[:], 0.0)
            # warmup psum shares the "ps" tag/shape so the pool fits 4 real
            # buffers in the 8 PSUM banks (4 x 2 banks, slot 0 doubles as
            # the warmup target before the rotation reaches it again).
            warm_ps = ppool.tile([128, OUT_W], DT, tag="ps", bufs=4)
            # many SMALL warmups (128 cols, ~160 ns each at the gated clock):
            # the first input tile's landing time varies run to run, and any
            # PE idle gap longer than the HAM hysteresis resets the ramp
            # accumulator.  Fine granularity keeps the eventual gap tiny and
            # an overshoot costs only ~60 ns per extra matmul.
            for _ in range(28):
                nc.tensor.matmul(
                    warm_ps[:, 0:128], dummy[:, 0:128], dummy[:, 0:128],
                    start=True, stop=True
                )

            w_e0 = w_tile[:, 0:64]
            w_em1 = w_tile[:, 64:128]
            w_o0 = w_tile[:, 128:192]
            w_om1 = w_tile[:, 192:256]
            w_op1 = w_tile[:, 256:320]

            # the out-DMA for big-batch bb is emitted one iteration LATE
            # (after bb+1's casts on the same scalar queue): an in-order
            # engine queue stalls on a waiting DMA, and a stalled head
            # blocks every ACT behind it, which delays PSUM buffer release
            # and starves the PE.  One big-batch of delay makes the wait
            # already-satisfied at queue head.
            pending_out = None
            for bb in range(N_BB):
                in_tile = ipool.tile([128, BB_W], IN_DT)
                if bb == 0:
                    # first tile split in two: the whole machine runs at the
                    # gated half clock until the PE ramp fires, so the first
                    # transfer is slow -- a half tile lets sb0's matmuls
                    # start ~2 us earlier (subtile deps cover the k=0 half).
                    nc.sync.dma_start(in_tile[:, 0:OUT_W], x_d[bb][:, 0:OUT_W])
                    nc.sync.dma_start(in_tile[:, OUT_W:BB_W], x_d[bb][:, OUT_W:BB_W])
                else:
                    nc.sync.dma_start(in_tile[:], x_d[bb])
                out_tile = opool.tile([128, BB_W], IN_DT)

                for k in range(2):
                    kb = k * OUT_W
                    # one 2-bank psum tile per sub-batch: even in cols 0-511
                    # (bank 0), odd in 512-1023 (bank 1)
                    ps = ppool.tile([128, OUT_W], DT, tag="ps", bufs=4)
                    ps_e = ps[:, 0:CG_W]
                    ps_o = ps[:, CG_W:OUT_W]

                    # window list per column group; interleave cg inner so
                    # the two PE column halves stay concurrently busy.
                    wins = []
                    for cg in range(2):
                        wins.append((ps_e, (0, HALF), w_e0, 0, HALF, True, False, cg))
                        wins.append((ps_e, (1, HALF - 1), w_em1, 0, HALF - 1, False, True, cg))
                        wins.append((ps_o, (0, HALF), w_o0, 0, HALF, True, False, cg))
                        wins.append((ps_o, (1, HALF - 1), w_om1, 0, HALF - 1, False, False, cg))
                        wins.append((ps_o, (0, HALF - 1), w_op1, 1, HALF - 1, False, True, cg))
                    order = [0, 5, 1, 6, 2, 7, 3, 8, 4, 9]  # cg0/cg1 interleaved
                    for idx in order:
                        psr, (jlo, jn), lhsT, rlo, rlen, st, sp, cg = wins[idx]
                        cgb = kb + cg * CG_W
                        rhs = in_tile[:, cgb:cgb + CG_W].rearrange(
                            "p (g w) -> p g w", w=HALF
                        )[:, :, rlo:rlo + rlen]
                        out_ap = psr[cg * 64:(cg + 1) * 64, :].rearrange(
                            "p (g w) -> p g w", w=HALF
                        )[:, :, jlo:jlo + jn]
                        nc.tensor.matmul(
                            out_ap,
                            lhsT,
                            rhs,
                            start=st,
                            stop=sp,
                            tile_position=(0, cg * 64),
                            skip_group_check=True,
                        )

                    # psum->sbuf bf16 cast, split 3:1 across DVE and
                    # Activation: one DVE cast of the full [128,1024] takes
                    # 1214 ns -- more than the 1047 ns PE floor per
                    # sub-batch, so it would pace the pipeline.  (gpsimd
                    # cannot read PSUM on hardware.)
                    nc.vector.tensor_copy(out_tile[:, kb:kb + 768], ps[:, 0:768])
                    nc.scalar.copy(out_tile[:, kb + 768:kb + OUT_W], ps[:, 768:OUT_W])
                    if bb >= N_BB - 2:
                        # stream the last two big-batches out per sub-batch
                        # on alternating queues: their drains overlap the
                        # last compute instead of serializing after it.
                        eng = nc.scalar if k == 0 else nc.sync
                        eng.dma_start(o_d[bb][:, kb:kb + OUT_W],
                                      out_tile[:, kb:kb + OUT_W])

                if pending_out is not None and bb < N_BB - 1:
                    nc.scalar.dma_start(o_d[bb - 1], pending_out[:])
                pending_out = out_tile

            # end-warmup: the clock domain gates to half speed ~3 us after
            # the PE goes idle, which halves the final out-DMA drain rate.
            # Six dummy matmuls (~2.3 us) keep the clock up until the last
            # transfer completes.  Fresh pool tile: slot rotation makes it
            # wait only on an already-drained sub-batch, not the final one.
            warm_end = ppool.tile([128, OUT_W], DT, tag="ps", bufs=4)
            for _ in range(10):
                nc.tensor.matmul(
                    warm_end[:, 0:512], dummy[:, 0:128], dummy[:], start=True, stop=True
                )
    nc.compile()
    return nc


def _host_pack(x: np.ndarray) -> np.ndarray:
    """FULL x (8192,64,64) f32 -> [N_CORES, N_SB, 128, IN_W] bf16.

    Partition dim = (s = w&1, h); free dim = (g: 32 images, jo: 32)."""
    xb = x.astype(NP_IN)
    v = xb.reshape(N_CORES, N_SB, SB_IMGS, IMG, HALF, 2)
    v = v.transpose(0, 1, 5, 3, 2, 4)  # [core, sb, s, h, g, jo]
    flat = v.reshape(N_CORES, N_SB, 128, SB_IMGS * HALF)
    # regroup pairs of sub-batches into big-batch tiles [128, 2048]
    bb = flat.reshape(N_CORES, N_BB, 2, 128, OUT_W).transpose(0, 1, 3, 2, 4)
    return np.ascontiguousarray(bb.reshape(N_CORES, N_BB, 128, BB_W))


def _host_unpack(tiles: np.ndarray) -> np.ndarray:
    """[N_CORES, N_SB, 128, OUT_W] -> (8192, 64, 64) f32.

    Partition = (cg, i); free = (t: even/odd, g: 16, jo: 32); j = 2*jo+t;
    img = sb*32 + cg*16 + g."""
    # split big-batch tiles [N_BB, 128, 2048] back into sub-batches
    tiles = tiles.reshape(N_CORES, N_BB, 128, 2, OUT_W).transpose(0, 1, 3, 2, 4)
    v = tiles.reshape(N_CORES, N_SB, 2, IMG, 2, CG_IMGS, HALF)
    v = v.transpose(0, 1, 2, 5, 3, 6, 4)  # [core, sb, cg, g, i, jo, t]
    return np.ascontiguousarray(v).reshape(N_IMAGES, IMG, IMG)


def kernel(x: np.ndarray, kernel: np.ndarray, _trace: bool = False) -> np.ndarray:
    global LAST_RESULTS
    x = np.ascontiguousarray(np.asarray(x, dtype=np.float32))
    n, c, h, w = x.shape
    assert (n, c, h, w) == (16, 512, 64, 64), x.shape

    shards = _host_pack(x.reshape(N_IMAGES, IMG, IMG))
    wts = _build_weights(kernel)
    in_maps = [{"x": shards[i], "wts": wts} for i in range(N_CORES)]

    nc = _bass_module()
    results = run_bass_kernel_spmd(
        nc, in_maps, core_ids=list(range(N_CORES)), trace=_trace
    )
    LAST_RESULTS = results

    tiles = np.stack([r["out"] for r in results.results]).astype(np.float32)
    out = _host_unpack(tiles)
    return np.ascontiguousarray(out.reshape(n, c, h, w)).astype(np.float32)
